# revision 18
# baseline (speedup 1.0000x reference)
"""CARAFE content-aware upsampling on 8 Trainium2 NeuronCores — v4.

Strategy (data parallel): 8 cores = 4 batch images x 2 row-halves
(32 low-res rows each, +2-row halo). Per core, fully fused in SBUF:
  A) y_down = conv1x1(x, w_down)+b_down        (PE, K=256 in 2 chunks)
  Z) zT = (w_out . x) transposed               (PE produces [col, ch] directly)
  B) enc = conv3x3(y_down, w_enc)              (PE, 9 shifted accum matmuls)
  C) mask = softmax over 25 taps               (PE transpose+group-sums via an
     augmented selector matmul, DVE reciprocal + normalize)
  D) out = sum_k zT[window] * mask  + b_out    (PE banded matmuls, K=36
     w-halves on alternating PE row-groups 0/64 + alternating PSUM banks so
     each LDWEIGHTS hides under the other group's stream)

v4 structure:
  - Banded masks go through a DRAM staging image (HW DMA cannot write SBUF
    diagonally): diagonal scatter maskv->bstage slot, canonical reload into
    a [128, 8, 1280] rotating-slot band tile (rows 0:36 = w-half 0 band,
    rows 64:100 = w-half 1 band).
  - Software-pipelined emission: C(kc) chains [selector matmul, normalize,
    2 scatters, 2 reloads] run 4 kc ahead of D(2kc-8..); A/B/Z staged as
    filler between D pairs.
  - Output: psum -> obuf copies fold b_out and deinterleave (w-half, w, p)
    -> hi-res (a, w, b) rows; 16 contiguous out-DMAs (4KB runs/partition).
"""

import sys
import functools
import numpy as np
from contextlib import ExitStack

for _p in ("/opt/trn_rl_repo",):
    if _p not in sys.path:
        sys.path.insert(0, _p)

import concourse.bass as bass
import concourse.bacc as bacc
import concourse.mybir as mybir
import concourse.tile as tile
from concourse.bass_utils import run_bass_kernel_spmd

NCORES = 8
FP = mybir.dt.float32
BF = mybir.dt.bfloat16
AF = mybir.ActivationFunctionType
ALU = mybir.AluOpType

NSLOT = 16


def _ap(base, offset_delta, dims):
    return bass.AP(tensor=base.tensor, offset=base.offset + offset_delta, ap=dims)


def _build_program(nc, serial=False):
    xs_d = nc.declare_dram_parameter("xs", [256, 36, 68], BF, isOutput=False)
    wdt_d = nc.declare_dram_parameter("wdt", [256, 128], BF, isOutput=False)
    wet_d = nc.declare_dram_parameter("wet", [128, 9, 100], BF, isOutput=False)
    wot_d = nc.declare_dram_parameter("wot", [256, 256], BF, isOutput=False)
    bd_d = nc.declare_dram_parameter("bd", [128, 1], FP, isOutput=False)
    be_d = nc.declare_dram_parameter("be", [100, 1], FP, isOutput=False)
    bo_d = nc.declare_dram_parameter("bo", [256, 1], FP, isOutput=False)
    saug_d = nc.declare_dram_parameter("saug", [100, 104], BF, isOutput=False)
    bstage = nc.declare_dram_parameter("bstage", [NSLOT, 68, 1280], BF, isOutput=False)
    edge_d = nc.declare_dram_parameter("edge", [1, 2], FP, isOutput=False)
    out_d = nc.declare_dram_parameter("out", [256, 64, 128], FP, isOutput=True)

    with tile.TileContext(nc) as tc:
        with ExitStack() as ctx:
            const = ctx.enter_context(tc.tile_pool(name="const", bufs=1))
            big = ctx.enter_context(tc.tile_pool(name="big", bufs=1))
            opool = ctx.enter_context(tc.tile_pool(name="opool", bufs=2))
            bpool = ctx.enter_context(tc.tile_pool(name="bpool", bufs=1))
            psA = ctx.enter_context(tc.tile_pool(name="psA", bufs=1, space="PSUM"))
            psZ = ctx.enter_context(tc.tile_pool(name="psZ", bufs=2, space="PSUM"))
            psBC = ctx.enter_context(tc.tile_pool(name="psBC", bufs=1, space="PSUM"))
            psD = ctx.enter_context(tc.tile_pool(name="psD", bufs=2, space="PSUM"))

            # ---- loads (sync + scalar queues) ----
            xa = big.tile([128, 36, 68], BF)
            xb = big.tile([128, 36, 68], BF)
            wdt = const.tile([128, 2, 128], BF)
            nc.sync.dma_start(
                out=_ap(wdt[:], 0, [[256, 128], [128, 2], [1, 128]]),
                in_=bass.AP(tensor=wdt_d, offset=0,
                            ap=[[128, 128], [128 * 128, 2], [1, 128]]),
            )
            bd = const.tile([128, 1], FP)
            nc.sync.dma_start(out=bd[:], in_=bd_d[:])
            edge = const.tile([128, 2], FP)
            nc.sync.dma_start(
                out=edge[:],
                in_=bass.AP(tensor=edge_d, offset=0, ap=[[0, 128], [1, 2]]),
            )
            for r0, r1 in ((0, 13), (13, 25), (25, 36)):
                nc.sync.dma_start(out=xa[:, r0:r1, :], in_=xs_d[0:128, r0:r1, :])
                nc.scalar.dma_start(out=xb[:, r0:r1, :], in_=xs_d[128:256, r0:r1, :])
            wet = const.tile([128, 9, 100], BF)
            nc.scalar.dma_start(out=wet[:], in_=wet_d[:])
            wot = const.tile([128, 2, 256], BF)
            nc.sync.dma_start(
                out=_ap(wot[:], 0, [[512, 128], [256, 2], [1, 256]]),
                in_=bass.AP(tensor=wot_d, offset=0,
                            ap=[[256, 128], [128 * 256, 2], [1, 256]]),
            )
            be = const.tile([100, 1], FP)
            nc.scalar.dma_start(out=be[:], in_=be_d[:])
            bo = const.tile([128, 2], FP)
            nc.scalar.dma_start(
                out=_ap(bo[:], 0, [[2, 128], [1, 2]]),
                in_=bass.AP(tensor=bo_d, offset=0, ap=[[1, 128], [128, 2]]),
            )
            saug = const.tile([100, 104], BF)
            nc.scalar.dma_start(out=saug[:], in_=saug_d[:])

            ydown = big.tile([128, 34, 66], BF)
            zt = big.tile([68, 36, 256], BF)
            zt2 = big.tile([128, 36, 256], BF)
            expv = big.tile([100, 32, 64], BF)
            maskv = big.tile([128, 16, 100], BF)
            inv = big.tile([128, 16, 4], FP)

            # ydown cols 0 and 65 are off-image: zero once; stage A then only
            # writes cols 1..64 so the zeros survive the interleaved schedule.
            nc.vector.memset(ydown[:, :, 0:1], 0.0)
            nc.vector.memset(ydown[:, :, 65:66], 0.0)

            # band tile (rotating per-h slots); the DRAM staging image the
            # diagonal scatter writes is a host-zeroed INPUT (off-band cells
            # stay zero forever: every scatter hits the same diagonal cells)
            band = bpool.tile([128, NSLOT, 1280], BF, name="band")

            # output staging: [ch-half][buf] -> 4 lo-res rows = 8 hi-res rows
            obufs = [
                [opool.tile([128, 4, 256], FP, tag=f"ob{half}", name=f"ob{half}_{b}")
                 for b in range(2)]
                for half in range(2)
            ]

            row_blocks = [(0, 6), (6, 12), (12, 18), (18, 24), (24, 30), (30, 34)]

            def stage_a(bi):
                r0, r1 = row_blocks[bi]
                nr = r1 - r0
                pa = psA.tile([128, 6, 66], FP, tag="A", name=f"pa{bi}")
                nc.tensor.matmul(
                    pa[:, 0:nr, :], wdt[:, 0, :], xa[:, 1 + r0 : 1 + r1, 1:67],
                    start=True, stop=False,
                )
                nc.tensor.matmul(
                    pa[:, 0:nr, :], wdt[:, 1, :], xb[:, 1 + r0 : 1 + r1, 1:67],
                    start=False, stop=True,
                )
                if r0 == 0:
                    nc.vector.tensor_scalar(
                        ydown[:, 0:1, 1:65], pa[:, 0:1, 1:65], bd[:], edge[:, 0:1],
                        op0=ALU.add, op1=ALU.mult,
                    )
                    nc.scalar.add(ydown[:, 1:6, 1:65], pa[:, 1:6, 1:65], add=bd[:])
                elif r1 == 34:
                    nc.vector.tensor_scalar(
                        ydown[:, 33:34, 1:65], pa[:, 3:4, 1:65], bd[:], edge[:, 1:2],
                        op0=ALU.add, op1=ALU.mult,
                    )
                    nc.scalar.add(ydown[:, 30:33, 1:65], pa[:, 0:3, 1:65], add=bd[:])
                else:
                    if bi % 2 == 0:
                        nc.vector.tensor_scalar(
                            ydown[:, r0:r1, 1:65], pa[:, 0:nr, 1:65], bd[:], None,
                            op0=ALU.add,
                        )
                    else:
                        nc.scalar.add(ydown[:, r0:r1, 1:65], pa[:, 0:nr, 1:65], add=bd[:])

            def stage_z2(g):
                # 2 rows per group; two sequential K=256 chains in one bank
                pz = psZ.tile([68, 2, 256], FP, tag="Z", name=f"pz{g}")
                for rr in range(2):
                    r = 2 * g + rr
                    nc.tensor.matmul(
                        pz[:, rr, :], xa[:, r, :], wot[:, 0, :], start=True, stop=False
                    )
                    nc.tensor.matmul(
                        pz[:, rr, :], xb[:, r, :], wot[:, 1, :], start=False, stop=True
                    )
                if g % 2 == 0:
                    nc.vector.tensor_copy(zt[:, 2 * g : 2 * g + 2, :], pz[:])
                else:
                    nc.scalar.copy(zt[:, 2 * g : 2 * g + 2, :], pz[:])

            def zt2_copy(r0, r1):
                nc.sync.dma_start(
                    out=zt2[64:100, r0:r1, :], in_=zt[32:68, r0:r1, :]
                )

            def stage_b(b4):
                pb = psBC.tile([100, 8, 64], FP, tag="BC", name=f"pb{b4}")
                k = 0
                for di in range(3):
                    for dj in range(3):
                        nc.tensor.matmul(
                            pb[:],
                            wet[:, 3 * di + dj, :],
                            ydown[:, di + 8 * b4 : di + 8 * b4 + 8, dj : dj + 64],
                            start=(k == 0), stop=(k == 8),
                        )
                        k += 1
                nc.scalar.activation(
                    expv[:, 8 * b4 : 8 * b4 + 8, :], pb[:], AF.Exp, bias=be[:]
                )

            expf = expv[:].rearrange("p a b -> p (a b)")

            def stage_c(kc):
                pc = psBC.tile([128, 104], FP, tag="BC", name=f"pc{kc}")
                nc.tensor.matmul(
                    pc[:],
                    expf[:, 128 * kc : 128 * (kc + 1)],
                    saug[:],
                    start=True, stop=True,
                )
                nc.vector.reciprocal(inv[:, kc, :], pc[:, 100:104])
                inv_b = _ap(inv[:], kc * 4, [[64, 128], [0, 25], [1, 4]])
                nc.vector.tensor_tensor(
                    maskv[:, kc, :].rearrange("p (k q) -> p k q", q=4),
                    pc[:, 0:100].rearrange("p (k q) -> p k q", q=4),
                    inv_b,
                    op=ALU.mult,
                )

            def scatter(h):
                # diagonal scatter into the DRAM staging slot for row h
                kc, hh = divmod(h, 2)
                slot = h % NSLOT
                srcm = _ap(
                    maskv[:], hh * 64 * 1600 + kc * 100,
                    [[1600, 64], [20, 5], [1, 20]],
                )
                dstm = bass.AP(
                    tensor=bstage, offset=slot * 68 * 1280,
                    ap=[[1300, 64], [1280, 5], [1, 20]],
                )
                nc.sync.dma_start(out=dstm, in_=srcm)

            def reload(kc, nkc=2):
                # bring back nkc kc's worth of slots (2*nkc rows): w-half 0
                # band rows on partitions 0:36, w-half 1 on partitions 64:100
                s = (2 * kc) % NSLOT
                ns = 2 * nkc
                srcr = bass.AP(
                    tensor=bstage, offset=s * 68 * 1280,
                    ap=[[1280, 36], [68 * 1280, ns], [1, 1280]],
                )
                nc.scalar.dma_start(out=band[0:36, s : s + ns, :], in_=srcr)
                srcr2 = bass.AP(
                    tensor=bstage, offset=s * 68 * 1280 + 32 * 1280,
                    ap=[[1280, 36], [68 * 1280, ns], [1, 1280]],
                )
                nc.scalar.dma_start(out=band[64:100, s : s + ns, :], in_=srcr2)

            def cchain(kc):
                stage_c(kc)
                scatter(2 * kc)
                scatter(2 * kc + 1)

            PITCH = NSLOT * 1280

            def stage_d(h):
                slot = h % NSLOT
                chunk, q = divmod(h, 4)
                pd = psD.tile([128, 1024], FP, tag="D", name=f"pd{h}")
                # 4 chains: (wh, half) -> bank wh, col block 128*half
                for half in range(2):
                    for i in range(5):
                        for wh in range(2):
                            rhs = _ap(
                                band[:],
                                64 * PITCH * wh + slot * 1280 + 640 * wh + 4 * i,
                                [[PITCH, 36], [20, 32], [1, 4]],
                            )
                            lhsT = (
                                zt[0:36, h + i, 128 * half : 128 * half + 128]
                                if wh == 0
                                else zt2[64:100, h + i, 128 * half : 128 * half + 128]
                            )
                            nc.tensor.matmul(
                                _ap(
                                    pd[:],
                                    512 * wh + 128 * half,
                                    [[1024, 128], [4, 32], [1, 4]],
                                ),
                                lhsT, rhs,
                                start=(i == 0), stop=(i == 4),
                            )
                for half in range(2):
                    # deinterleave (wh, w, p) -> hi-res (a, w, b) + bias
                    ob = obufs[half][chunk % 2]
                    boh = bo[:, half : half + 1]
                    for wh in range(2):
                        srcv = _ap(
                            pd[:], 512 * wh + 128 * half,
                            [[1024, 128], [2, 2], [4, 32], [1, 2]],
                        )
                        dstv = _ap(
                            ob[:], q * 256 + 64 * wh,
                            [[1024, 128], [128, 2], [2, 32], [1, 2]],
                        )
                        if (h + wh) % 2 == 0:
                            nc.vector.tensor_scalar(dstv, srcv, boh, None, op0=ALU.add)
                        else:
                            nc.scalar.add(dstv, srcv, add=boh)
                if chunk == 7 and q in (1, 3):
                    # last chunk: fire half-chunks early to shrink the tail
                    part = q // 2
                    for half in range(2):
                        ob = obufs[half][chunk % 2]
                        dst = bass.AP(
                            tensor=out_d,
                            offset=half * 128 * 8192 + chunk * 8 * 128
                            + part * 4 * 128,
                            ap=[[8192, 128], [1, 512]],
                        )
                        src = _ap(ob[:], part * 512, [[1024, 128], [1, 512]])
                        nc.gpsimd.dma_start(out=dst, in_=src)
                elif q == 3:
                    for half in range(2):
                        ob = obufs[half][chunk % 2]
                        dst = bass.AP(
                            tensor=out_d,
                            offset=half * 128 * 8192 + chunk * 8 * 128,
                            ap=[[8192, 128], [1, 1024]],
                        )
                        src = _ap(ob[:], 0, [[1024, 128], [1, 1024]])
                        nc.gpsimd.dma_start(out=dst, in_=src)

            # ---- emission schedule ----
            def zp(k):
                stage_z2(2 * k)
                stage_z2(2 * k + 1)

            if serial:
                for bi in range(6):
                    stage_a(bi)
                for g in range(18):
                    stage_z2(g)
                zt2_copy(0, 36)
                for b4 in range(4):
                    stage_b(b4)
                for kc in range(16):
                    stage_c(kc)
                    scatter(2 * kc)
                    scatter(2 * kc + 1)
                    reload(kc, nkc=1)
                    stage_d(2 * kc)
                    stage_d(2 * kc + 1)
            else:
                stage_a(0)
                zp(0)
                stage_a(1)
                stage_b(0)
                zp(1)
                cchain(0)
                zp(2)
                zt2_copy(0, 12)
                cchain(1)
                zp(3)
                stage_a(2)
                cchain(2)
                reload(0)
                stage_a(3)
                zp(4)
                cchain(3)
                stage_b(1)
                cchain(4)
                reload(2)

                filler = {
                    6: [lambda: zp(5), lambda: stage_a(4)],
                    7: [lambda: stage_a(5), lambda: stage_b(2)],
                    8: [lambda: zp(6), lambda: zt2_copy(12, 24)],
                    9: [lambda: zp(7)],
                    10: [lambda: stage_b(3)],
                    11: [lambda: zp(8)],
                    13: [lambda: zt2_copy(24, 36)],
                }
                for kc in range(5, 16):
                    cchain(kc)
                    if kc % 2 == 0:
                        reload(kc - 2)
                    stage_d(2 * kc - 10)
                    stage_d(2 * kc - 9)
                    for f in filler.get(kc, []):
                        f()
                reload(14)
                for h in range(22, 32):
                    stage_d(h)

    nc.compile()
    return nc


@functools.lru_cache(maxsize=2)
def _build(num_devices=NCORES, serial=False):
    nc = bacc.Bacc("TRN2", target_bir_lowering=False, debug=False,
                   num_devices=num_devices)
    return _build_program(nc, serial=serial)


def _host_prep(x, w_down, b_down, w_enc, b_enc, w_out, b_out, ncores=NCORES):
    import ml_dtypes

    bft = ml_dtypes.bfloat16
    x = np.asarray(x, np.float32)
    xp = np.pad(x, [(0, 0), (0, 0), (2, 2), (2, 2)]).astype(bft)
    wdt = np.ascontiguousarray(np.asarray(w_down, np.float32)[:, :, 0, 0].T.astype(bft))
    wet = np.ascontiguousarray(
        np.asarray(w_enc, np.float32).transpose(1, 2, 3, 0).reshape(128, 9, 100)
    ).astype(bft)
    wot = np.ascontiguousarray(np.asarray(w_out, np.float32)[:, :, 0, 0].T.astype(bft))
    bd = np.asarray(b_down, np.float32).reshape(128, 1)
    be = np.asarray(b_enc, np.float32).reshape(100, 1)
    bo = np.asarray(b_out, np.float32).reshape(256, 1)
    # saug: permuted identity (e=(i5,j5,p4) -> e'=(j5,i5,p4)) + 4 group-sum cols
    saug = np.zeros((100, 104), bft)
    for i in range(5):
        for j in range(5):
            for p in range(4):
                saug[(i * 5 + j) * 4 + p, j * 20 + i * 4 + p] = 1.0
    for e in range(100):
        saug[e, 100 + e % 4] = 1.0
    in_maps = []
    for c in range(ncores):
        n, hh = c // 2, c % 2
        xs = np.ascontiguousarray(xp[n, :, hh * 32 : hh * 32 + 36, :])
        edge = np.array(
            [[0.0 if hh == 0 else 1.0, 0.0 if hh == 1 else 1.0]], np.float32
        )
        in_maps.append(
            dict(xs=xs, wdt=wdt, wet=wet, wot=wot, bd=bd, be=be, bo=bo,
                 saug=saug, edge=edge,
                 bstage=np.zeros((16, 68, 1280), bft))
        )
    return in_maps


last_exec_time_ns = None


def kernel(x, w_down, b_down, w_enc, b_enc, w_out, b_out):
    global last_exec_time_ns
    nc = _build()
    in_maps = _host_prep(x, w_down, b_down, w_enc, b_enc, w_out, b_out)
    res = run_bass_kernel_spmd(nc, in_maps, list(range(NCORES)))
    last_exec_time_ns = res.exec_time_ns
    out = np.empty((4, 256, 128, 128), np.float32)
    for c in range(NCORES):
        n, hh = c // 2, c % 2
        out[n, :, hh * 64 : (hh + 1) * 64, :] = res.results[c]["out"]
    return out


# revision 20
# speedup vs baseline: 1.1346x; 1.1346x over previous
"""CARAFE content-aware upsampling on 8 Trainium2 NeuronCores — v4.

Strategy (data parallel): 8 cores = 4 batch images x 2 row-halves
(32 low-res rows each, +2-row halo). Per core, fully fused in SBUF:
  A) y_down = conv1x1(x, w_down)+b_down        (PE, K=256 in 2 chunks)
  Z) zT = (w_out . x) transposed               (PE produces [col, ch] directly)
  B) enc = conv3x3(y_down, w_enc)              (PE, 9 shifted accum matmuls)
  C) mask = softmax over 25 taps               (PE transpose+group-sums via an
     augmented selector matmul, DVE reciprocal + normalize)
  D) out = sum_k zT[window] * mask  + b_out    (PE banded matmuls, K=36
     w-halves on alternating PE row-groups 0/64 + alternating PSUM banks so
     each LDWEIGHTS hides under the other group's stream)

v4 structure:
  - Banded masks go through a DRAM staging image (HW DMA cannot write SBUF
    diagonally): diagonal scatter maskv->bstage slot, canonical reload into
    a [128, 8, 1280] rotating-slot band tile (rows 0:36 = w-half 0 band,
    rows 64:100 = w-half 1 band).
  - Software-pipelined emission: C(kc) chains [selector matmul, normalize,
    2 scatters, 2 reloads] run 4 kc ahead of D(2kc-8..); A/B/Z staged as
    filler between D pairs.
  - Output: psum -> obuf copies fold b_out and deinterleave (w-half, w, p)
    -> hi-res (a, w, b) rows; 16 contiguous out-DMAs (4KB runs/partition).
"""

import sys
import functools
import numpy as np
from contextlib import ExitStack

for _p in ("/opt/trn_rl_repo",):
    if _p not in sys.path:
        sys.path.insert(0, _p)

import concourse.bass as bass
import concourse.bacc as bacc
import concourse.mybir as mybir
import concourse.tile as tile
from concourse.bass_utils import run_bass_kernel_spmd

NCORES = 8
FP = mybir.dt.float32
BF = mybir.dt.bfloat16
AF = mybir.ActivationFunctionType
ALU = mybir.AluOpType

NSLOT = 16


def _ap(base, offset_delta, dims):
    return bass.AP(tensor=base.tensor, offset=base.offset + offset_delta, ap=dims)


def _build_program(nc, serial=False):
    xs_d = nc.declare_dram_parameter("xs", [256, 36, 68], BF, isOutput=False)
    wdt_d = nc.declare_dram_parameter("wdt", [256, 128], BF, isOutput=False)
    wet_d = nc.declare_dram_parameter("wet", [128, 9, 100], BF, isOutput=False)
    wot_d = nc.declare_dram_parameter("wot", [256, 256], BF, isOutput=False)
    bd_d = nc.declare_dram_parameter("bd", [128, 1], FP, isOutput=False)
    be_d = nc.declare_dram_parameter("be", [100, 1], FP, isOutput=False)
    bo_d = nc.declare_dram_parameter("bo", [256, 1], FP, isOutput=False)
    saug_d = nc.declare_dram_parameter("saug", [100, 104], BF, isOutput=False)
    bstage = nc.declare_dram_parameter("bstage", [NSLOT, 68, 1280], BF, isOutput=False)
    edge_d = nc.declare_dram_parameter("edge", [1, 2], FP, isOutput=False)
    out_d = nc.declare_dram_parameter("out", [256, 64, 128], FP, isOutput=True)

    with tile.TileContext(nc) as tc:
        with ExitStack() as ctx:
            const = ctx.enter_context(tc.tile_pool(name="const", bufs=1))
            big = ctx.enter_context(tc.tile_pool(name="big", bufs=1))
            opool = ctx.enter_context(tc.tile_pool(name="opool", bufs=2))
            bpool = ctx.enter_context(tc.tile_pool(name="bpool", bufs=1))
            psA = ctx.enter_context(tc.tile_pool(name="psA", bufs=1, space="PSUM"))
            psZ = ctx.enter_context(tc.tile_pool(name="psZ", bufs=2, space="PSUM"))
            psBC = ctx.enter_context(tc.tile_pool(name="psBC", bufs=1, space="PSUM"))
            psD = ctx.enter_context(tc.tile_pool(name="psD", bufs=2, space="PSUM"))

            # ---- loads (sync + scalar queues) ----
            xa = big.tile([128, 36, 68], BF)
            xb = big.tile([128, 36, 68], BF)
            wdt = const.tile([128, 2, 128], BF)
            nc.sync.dma_start(
                out=_ap(wdt[:], 0, [[256, 128], [128, 2], [1, 128]]),
                in_=bass.AP(tensor=wdt_d, offset=0,
                            ap=[[128, 128], [128 * 128, 2], [1, 128]]),
            )
            bd = const.tile([128, 1], FP)
            nc.sync.dma_start(out=bd[:], in_=bd_d[:])
            edge = const.tile([128, 2], FP)
            nc.sync.dma_start(
                out=edge[:],
                in_=bass.AP(tensor=edge_d, offset=0, ap=[[0, 128], [1, 2]]),
            )
            for r0, r1 in ((0, 13), (13, 25), (25, 36)):
                nc.sync.dma_start(out=xa[:, r0:r1, :], in_=xs_d[0:128, r0:r1, :])
                nc.scalar.dma_start(out=xb[:, r0:r1, :], in_=xs_d[128:256, r0:r1, :])
            wet = const.tile([128, 9, 100], BF)
            nc.scalar.dma_start(out=wet[:], in_=wet_d[:])
            wot = const.tile([128, 2, 256], BF)
            nc.sync.dma_start(
                out=_ap(wot[:], 0, [[512, 128], [256, 2], [1, 256]]),
                in_=bass.AP(tensor=wot_d, offset=0,
                            ap=[[256, 128], [128 * 256, 2], [1, 256]]),
            )
            be = const.tile([100, 1], FP)
            nc.scalar.dma_start(out=be[:], in_=be_d[:])
            bo = const.tile([128, 2], FP)
            nc.scalar.dma_start(
                out=_ap(bo[:], 0, [[2, 128], [1, 2]]),
                in_=bass.AP(tensor=bo_d, offset=0, ap=[[1, 128], [128, 2]]),
            )
            saug = const.tile([100, 104], BF)
            nc.scalar.dma_start(out=saug[:], in_=saug_d[:])

            ydown = big.tile([128, 34, 66], BF)
            zt = big.tile([68, 36, 256], BF)
            zt2 = big.tile([128, 36, 256], BF)
            expv = big.tile([100, 32, 64], BF)
            maskv = big.tile([128, 16, 100], BF)
            inv = big.tile([128, 16, 4], FP)

            # ydown cols 0 and 65 are off-image: zero once; stage A then only
            # writes cols 1..64 so the zeros survive the interleaved schedule.
            nc.vector.memset(ydown[:, :, 0:1], 0.0)
            nc.vector.memset(ydown[:, :, 65:66], 0.0)

            # band tile (rotating per-h slots); the DRAM staging image the
            # diagonal scatter writes is a host-zeroed INPUT (off-band cells
            # stay zero forever: every scatter hits the same diagonal cells)
            band = bpool.tile([128, NSLOT, 1280], BF, name="band")

            # output staging: [ch-half][buf] -> 4 lo-res rows = 8 hi-res rows
            obufs = [
                [opool.tile([128, 4, 256], FP, tag=f"ob{half}", name=f"ob{half}_{b}")
                 for b in range(2)]
                for half in range(2)
            ]

            row_blocks = [(0, 6), (6, 12), (12, 18), (18, 24), (24, 30), (30, 34)]

            def stage_a(bi):
                r0, r1 = row_blocks[bi]
                nr = r1 - r0
                pa = psA.tile([128, 6, 66], FP, tag="A", name=f"pa{bi}")
                nc.tensor.matmul(
                    pa[:, 0:nr, :], wdt[:, 0, :], xa[:, 1 + r0 : 1 + r1, 1:67],
                    start=True, stop=False,
                )
                nc.tensor.matmul(
                    pa[:, 0:nr, :], wdt[:, 1, :], xb[:, 1 + r0 : 1 + r1, 1:67],
                    start=False, stop=True,
                )
                if r0 == 0:
                    nc.vector.tensor_scalar(
                        ydown[:, 0:1, 1:65], pa[:, 0:1, 1:65], bd[:], edge[:, 0:1],
                        op0=ALU.add, op1=ALU.mult,
                    )
                    nc.scalar.add(ydown[:, 1:6, 1:65], pa[:, 1:6, 1:65], add=bd[:])
                elif r1 == 34:
                    nc.vector.tensor_scalar(
                        ydown[:, 33:34, 1:65], pa[:, 3:4, 1:65], bd[:], edge[:, 1:2],
                        op0=ALU.add, op1=ALU.mult,
                    )
                    nc.scalar.add(ydown[:, 30:33, 1:65], pa[:, 0:3, 1:65], add=bd[:])
                else:
                    if bi % 2 == 0:
                        nc.vector.tensor_scalar(
                            ydown[:, r0:r1, 1:65], pa[:, 0:nr, 1:65], bd[:], None,
                            op0=ALU.add,
                        )
                    else:
                        nc.scalar.add(ydown[:, r0:r1, 1:65], pa[:, 0:nr, 1:65], add=bd[:])

            def stage_z2(g):
                # 2 rows per group; two sequential K=256 chains in one bank
                pz = psZ.tile([68, 2, 256], FP, tag="Z", name=f"pz{g}")
                for rr in range(2):
                    r = 2 * g + rr
                    nc.tensor.matmul(
                        pz[:, rr, :], xa[:, r, :], wot[:, 0, :], start=True, stop=False
                    )
                    nc.tensor.matmul(
                        pz[:, rr, :], xb[:, r, :], wot[:, 1, :], start=False, stop=True
                    )
                if g % 2 == 0:
                    nc.vector.tensor_copy(zt[:, 2 * g : 2 * g + 2, :], pz[:])
                else:
                    nc.scalar.copy(zt[:, 2 * g : 2 * g + 2, :], pz[:])

            def zt2_copy(r0, r1):
                nc.sync.dma_start(
                    out=zt2[64:100, r0:r1, :], in_=zt[32:68, r0:r1, :]
                )

            def stage_b(b4):
                pb = psBC.tile([100, 8, 64], FP, tag="BC", name=f"pb{b4}")
                k = 0
                for di in range(3):
                    for dj in range(3):
                        nc.tensor.matmul(
                            pb[:],
                            wet[:, 3 * di + dj, :],
                            ydown[:, di + 8 * b4 : di + 8 * b4 + 8, dj : dj + 64],
                            start=(k == 0), stop=(k == 8),
                        )
                        k += 1
                nc.scalar.activation(
                    expv[:, 8 * b4 : 8 * b4 + 8, :], pb[:], AF.Exp, bias=be[:]
                )

            expf = expv[:].rearrange("p a b -> p (a b)")

            def stage_c(kc):
                pc = psBC.tile([128, 104], FP, tag="BC", name=f"pc{kc}")
                nc.tensor.matmul(
                    pc[:],
                    expf[:, 128 * kc : 128 * (kc + 1)],
                    saug[:],
                    start=True, stop=True,
                )
                nc.vector.reciprocal(inv[:, kc, :], pc[:, 100:104])
                inv_b = _ap(inv[:], kc * 4, [[64, 128], [0, 25], [1, 4]])
                nc.vector.tensor_tensor(
                    maskv[:, kc, :].rearrange("p (k q) -> p k q", q=4),
                    pc[:, 0:100].rearrange("p (k q) -> p k q", q=4),
                    inv_b,
                    op=ALU.mult,
                )

            def scatter(h):
                # diagonal scatter into the DRAM staging slot for row h
                kc, hh = divmod(h, 2)
                slot = h % NSLOT
                srcm = _ap(
                    maskv[:], hh * 64 * 1600 + kc * 100,
                    [[1600, 64], [20, 5], [1, 20]],
                )
                dstm = bass.AP(
                    tensor=bstage, offset=slot * 68 * 1280,
                    ap=[[1300, 64], [1280, 5], [1, 20]],
                )
                nc.sync.dma_start(out=dstm, in_=srcm)

            def reload(kc, nkc=2):
                # bring back nkc kc's worth of slots (2*nkc rows): w-half 0
                # band rows on partitions 0:36, w-half 1 on partitions 64:100
                s = (2 * kc) % NSLOT
                ns = 2 * nkc
                srcr = bass.AP(
                    tensor=bstage, offset=s * 68 * 1280,
                    ap=[[1280, 36], [68 * 1280, ns], [1, 1280]],
                )
                nc.scalar.dma_start(out=band[0:36, s : s + ns, :], in_=srcr)
                srcr2 = bass.AP(
                    tensor=bstage, offset=s * 68 * 1280 + 32 * 1280,
                    ap=[[1280, 36], [68 * 1280, ns], [1, 1280]],
                )
                nc.scalar.dma_start(out=band[64:100, s : s + ns, :], in_=srcr2)

            def cchain(kc):
                stage_c(kc)
                scatter(2 * kc)
                scatter(2 * kc + 1)

            PITCH = NSLOT * 1280

            def stage_d(h):
                slot = h % NSLOT
                chunk, q = divmod(h, 4)
                pd = psD.tile([128, 1024], FP, tag="D", name=f"pd{h}")
                # 4 chains: (wh, half) -> bank wh, col block 128*half
                for half in range(2):
                    for i in range(5):
                        for wh in range(2):
                            rhs = _ap(
                                band[:],
                                64 * PITCH * wh + slot * 1280 + 640 * wh + 4 * i,
                                [[PITCH, 36], [20, 32], [1, 4]],
                            )
                            lhsT = (
                                zt[0:36, h + i, 128 * half : 128 * half + 128]
                                if wh == 0
                                else zt2[64:100, h + i, 128 * half : 128 * half + 128]
                            )
                            nc.tensor.matmul(
                                _ap(
                                    pd[:],
                                    512 * wh + 128 * half,
                                    [[1024, 128], [4, 32], [1, 4]],
                                ),
                                lhsT, rhs,
                                start=(i == 0), stop=(i == 4),
                            )
                for half in range(2):
                    # deinterleave (wh, w, p) -> hi-res (a, w, b) + bias
                    ob = obufs[half][chunk % 2]
                    boh = bo[:, half : half + 1]
                    for wh in range(2):
                        srcv = _ap(
                            pd[:], 512 * wh + 128 * half,
                            [[1024, 128], [2, 2], [4, 32], [1, 2]],
                        )
                        dstv = _ap(
                            ob[:], q * 256 + 64 * wh,
                            [[1024, 128], [128, 2], [2, 32], [1, 2]],
                        )
                        if (h + wh) % 2 == 0:
                            nc.vector.tensor_scalar(dstv, srcv, boh, None, op0=ALU.add)
                        else:
                            nc.scalar.add(dstv, srcv, add=boh)
                if chunk == 7 and q in (1, 3):
                    # last chunk: fire half-chunks early to shrink the tail
                    part = q // 2
                    for half in range(2):
                        ob = obufs[half][chunk % 2]
                        dst = bass.AP(
                            tensor=out_d,
                            offset=half * 128 * 8192 + chunk * 8 * 128
                            + part * 4 * 128,
                            ap=[[8192, 128], [1, 512]],
                        )
                        src = _ap(ob[:], part * 512, [[1024, 128], [1, 512]])
                        nc.gpsimd.dma_start(out=dst, in_=src)
                elif q == 3:
                    for half in range(2):
                        ob = obufs[half][chunk % 2]
                        dst = bass.AP(
                            tensor=out_d,
                            offset=half * 128 * 8192 + chunk * 8 * 128,
                            ap=[[8192, 128], [1, 1024]],
                        )
                        src = _ap(ob[:], 0, [[1024, 128], [1, 1024]])
                        nc.gpsimd.dma_start(out=dst, in_=src)

            # ---- emission schedule ----
            def zp(k):
                stage_z2(2 * k)
                stage_z2(2 * k + 1)

            if serial:
                for bi in range(6):
                    stage_a(bi)
                for g in range(18):
                    stage_z2(g)
                zt2_copy(0, 36)
                for b4 in range(4):
                    stage_b(b4)
                for kc in range(16):
                    stage_c(kc)
                    scatter(2 * kc)
                    scatter(2 * kc + 1)
                    reload(kc, nkc=1)
                    stage_d(2 * kc)
                    stage_d(2 * kc + 1)
            else:
                stage_a(0)
                zp(0)
                stage_a(1)
                stage_b(0)
                zp(1)
                cchain(0)
                zp(2)
                zt2_copy(0, 12)
                cchain(1)
                zp(3)
                stage_a(2)
                cchain(2)
                reload(0)
                stage_a(3)
                zp(4)
                cchain(3)
                stage_b(1)
                cchain(4)
                reload(2)

                filler = {
                    6: [lambda: zp(5), lambda: stage_a(4)],
                    7: [lambda: stage_a(5), lambda: stage_b(2)],
                    8: [lambda: zp(6), lambda: zt2_copy(12, 24)],
                    9: [lambda: zp(7)],
                    10: [lambda: stage_b(3)],
                    11: [lambda: zp(8)],
                    13: [lambda: zt2_copy(24, 36)],
                }
                for kc in range(5, 16):
                    stage_d(2 * kc - 10)
                    stage_d(2 * kc - 9)
                    for f in filler.get(kc, []):
                        f()
                    cchain(kc)
                    if kc % 2 == 0:
                        reload(kc - 2)
                reload(14)
                for h in range(22, 32):
                    stage_d(h)

    nc.compile()
    return nc


@functools.lru_cache(maxsize=2)
def _build(num_devices=NCORES, serial=False):
    nc = bacc.Bacc("TRN2", target_bir_lowering=False, debug=False,
                   num_devices=num_devices)
    return _build_program(nc, serial=serial)


def _host_prep(x, w_down, b_down, w_enc, b_enc, w_out, b_out, ncores=NCORES):
    import ml_dtypes

    bft = ml_dtypes.bfloat16
    x = np.asarray(x, np.float32)
    xp = np.pad(x, [(0, 0), (0, 0), (2, 2), (2, 2)]).astype(bft)
    wdt = np.ascontiguousarray(np.asarray(w_down, np.float32)[:, :, 0, 0].T.astype(bft))
    wet = np.ascontiguousarray(
        np.asarray(w_enc, np.float32).transpose(1, 2, 3, 0).reshape(128, 9, 100)
    ).astype(bft)
    wot = np.ascontiguousarray(np.asarray(w_out, np.float32)[:, :, 0, 0].T.astype(bft))
    bd = np.asarray(b_down, np.float32).reshape(128, 1)
    be = np.asarray(b_enc, np.float32).reshape(100, 1)
    bo = np.asarray(b_out, np.float32).reshape(256, 1)
    # saug: permuted identity (e=(i5,j5,p4) -> e'=(j5,i5,p4)) + 4 group-sum cols
    saug = np.zeros((100, 104), bft)
    for i in range(5):
        for j in range(5):
            for p in range(4):
                saug[(i * 5 + j) * 4 + p, j * 20 + i * 4 + p] = 1.0
    for e in range(100):
        saug[e, 100 + e % 4] = 1.0
    in_maps = []
    for c in range(ncores):
        n, hh = c // 2, c % 2
        xs = np.ascontiguousarray(xp[n, :, hh * 32 : hh * 32 + 36, :])
        edge = np.array(
            [[0.0 if hh == 0 else 1.0, 0.0 if hh == 1 else 1.0]], np.float32
        )
        in_maps.append(
            dict(xs=xs, wdt=wdt, wet=wet, wot=wot, bd=bd, be=be, bo=bo,
                 saug=saug, edge=edge,
                 bstage=np.zeros((16, 68, 1280), bft))
        )
    return in_maps


last_exec_time_ns = None


def kernel(x, w_down, b_down, w_enc, b_enc, w_out, b_out):
    global last_exec_time_ns
    nc = _build()
    in_maps = _host_prep(x, w_down, b_down, w_enc, b_enc, w_out, b_out)
    res = run_bass_kernel_spmd(nc, in_maps, list(range(NCORES)))
    last_exec_time_ns = res.exec_time_ns
    out = np.empty((4, 256, 128, 128), np.float32)
    for c in range(NCORES):
        n, hh = c // 2, c % 2
        out[n, :, hh * 64 : (hh + 1) * 64, :] = res.results[c]["out"]
    return out
